# revision 5
# baseline (speedup 1.0000x reference)
"""Causal attention (DS_FullAttention) Trainium2 Bass kernel.

Problem: B=4, H=8, L=S=2048, E=64 causal attention with a per-batch
exp(tau) de-stationarization rescale, fp32 I/O.

Sharding: the 32 (b, h) pairs are independent; each of the 8 cores gets 4
pairs. Inside a core, per pair:
  - scoresT[s, q] = K^T Q computed transposed (s on PSUM partitions) so the
    A @ V contraction (over s) needs no on-chip transpose of A.
  - softmax exp is SPLIT across two engines: the scalar (ACT) engine runs
    table-based Exp with the per-pair scale folded into the ACTIVATE scale
    operand; the vector (DVE) engine runs a one-op Schraudolph exp
    (int16 = score * K + M, bits reinterpreted as fp16) for a tunable
    share of the off-diagonal chunks.  Max-subtraction is skipped; scores
    are O(8) so fp16 exp is safe.
  - V carries an appended ones column, so the A @ V matmul also produces
    the softmax denominators in PSUM partition 0.
  - causal masking: block-level (never compute s-chunks above the
    diagonal) + a triangular fp16 mask multiply on diagonal 128x128 chunks
    (gpsimd engine; DVE for pair 0 while gpsimd warms up).
  - normalization: reciprocal of the PSUM denominator row on DVE,
    partition-broadcast on gpsimd, one PSUM-sourced multiply on DVE
    writing fp16, fp16 output DMA (host upcasts to fp32).

Matmuls run in fp16 (measured end-to-end error ~5e-4 vs the fp32
reference; fp16 hides PE weight loads, fp32/f32r cannot).
"""

import sys

if "/opt/trn_rl_repo" not in sys.path:
    sys.path.insert(0, "/opt/trn_rl_repo")

import numpy as np

import concourse.bass as bass
import concourse.mybir as mybir
import concourse.tile as tile
from concourse import bacc, bass_utils

B, L, S, H, E = 4, 2048, 2048, 8, 64
P = 128
NCORES = 8
PAIRS_PER_CORE = (B * H) // NCORES  # 4
NQB = L // 512  # 4 q-superblocks of 512
NSC = S // P  # 16 s-chunks of 128
E2 = E + 1  # V plus ones column
EXP_GROUP = 3  # s-chunks exp'd per instruction (3 PSUM banks)

# Schraudolph fp16 exp constants: i16 = round(s*scale*K16 + M16); bits as
# fp16 give exp(s*scale) with ~3% max rel err (C=44 minimizes max rel err).
K16 = 2.0**10 / np.log(2.0)
M16 = 15.0 * 2**10 - 44.0
# Fraction of off-diagonal exp columns pushed to the DVE engine.
DVE_FRAC = 0.55

f32 = mybir.dt.float32
fp16 = mybir.dt.float16
i16 = mybir.dt.int16
Exp = mybir.ActivationFunctionType.Exp
Mult = mybir.AluOpType.mult
Add = mybir.AluOpType.add

_PROGRAM_CACHE = {}


def _build_program():
    if "nc" in _PROGRAM_CACHE:
        return _PROGRAM_CACHE["nc"]

    nc = bacc.Bacc(
        "TRN2",
        target_bir_lowering=False,
        debug=False,
        enable_asserts=False,
        num_devices=NCORES,
    )
    qt_d = nc.dram_tensor("qt", [PAIRS_PER_CORE, P, L], fp16, kind="ExternalInput")
    kt_d = nc.dram_tensor("kt", [PAIRS_PER_CORE, P, L], fp16, kind="ExternalInput")
    vp_d = nc.dram_tensor(
        "vp", [PAIRS_PER_CORE, P, NSC, E2], fp16, kind="ExternalInput"
    )
    tri_d = nc.dram_tensor("tri", [P, P], fp16, kind="ExternalInput")
    scl_d = nc.dram_tensor("scl", [P, PAIRS_PER_CORE], f32, kind="ExternalInput")
    sck_d = nc.dram_tensor("sck", [P, PAIRS_PER_CORE], f32, kind="ExternalInput")
    o_d = nc.dram_tensor("o", [PAIRS_PER_CORE, E, L], fp16, kind="ExternalOutput")

    with tile.TileContext(nc) as tc:
        with (
            tc.tile_pool(name="const", bufs=1) as const,
            tc.tile_pool(name="qk", bufs=2) as qk,
            tc.tile_pool(name="atp", bufs=5) as atp,
            tc.tile_pool(name="stg", bufs=3) as stg,
            tc.tile_pool(name="psS", bufs=2, space="PSUM") as psS,
            tc.tile_pool(name="psO", bufs=2, space="PSUM") as psO,
        ):
            # constants first so their DMAs lead the queue
            tri_t = const.tile([P, P], fp16)
            nc.sync.dma_start(tri_t[:], tri_d[:])
            scl_t = const.tile([P, PAIRS_PER_CORE], f32)
            nc.sync.dma_start(scl_t[:], scl_d[:])
            sck_t = const.tile([P, PAIRS_PER_CORE], f32)
            nc.sync.dma_start(sck_t[:], sck_d[:])

            # ACT warm-up: pull the exp table load under the input DMAs
            # (no gpsimd dependency -- ACT zeroes its own scratch).
            wu = const.tile([P, 16], f32)
            nc.scalar.memzero(wu[:])
            nc.scalar.activation(wu[:], wu[:], Exp, scale=1.0)
            # gpsimd warm-up: absorb its slow first-instruction init early.
            wg = const.tile([P, 16], f32)
            nc.gpsimd.memset(wg[:], 0.0)

            # mm2 + block epilogues run two exp-groups behind (RAW decoupling)
            pending = []

            def flush(depth=0):
                while len(pending) > depth:
                    pending.pop(0)()

            # greedy ACT/DVE off-diagonal exp balancing (estimated busy ns);
            # DVE starts pre-loaded with its epilogue + pair-0 mask work.
            eng_ns = {"act": 0.0, "dve": 21000.0 + 3200.0}

            for p in range(PAIRS_PER_CORE):
                qt_t = qk.tile([P, L], fp16, tag="qt")
                kt_t = qk.tile([P, L], fp16, tag="kt")
                vp_t = qk.tile([P, NSC, E2], fp16, tag="vp")
                if p == 0:  # small first slices so group 0 starts early
                    nc.sync.dma_start(kt_t[:, 0:512], kt_d[p][:, 0:512])
                    nc.sync.dma_start(qt_t[:, 0:512], qt_d[p][:, 0:512])
                    nc.sync.dma_start(vp_t[:, 0:4, :], vp_d[p][:, 0:4, :])
                    nc.sync.dma_start(kt_t[:, 512:L], kt_d[p][:, 512:L])
                    nc.sync.dma_start(qt_t[:, 512:L], qt_d[p][:, 512:L])
                    nc.sync.dma_start(vp_t[:, 4:NSC, :], vp_d[p][:, 4:NSC, :])
                else:
                    nc.sync.dma_start(qt_t[:], qt_d[p])
                    nc.sync.dma_start(kt_t[:], kt_d[p])
                    nc.sync.dma_start(vp_t[:], vp_d[p])

                # last pair runs its q-superblocks big-to-small so the tail
                # ends on the cheapest superblock (iq=0).
                iq_order = (
                    range(NQB) if p < PAIRS_PER_CORE - 1 else range(NQB - 1, -1, -1)
                )
                for iq in iq_order:
                    q0 = 512 * iq
                    njs = 4 * iq + 4  # s-chunks this q-superblock touches
                    po = psO.tile([P, 512], f32, tag="po")

                    def mk_mm2(js, at, po=po, vp_t=vp_t, iq=iq, njs=njs,
                               p=p, q0=q0):
                        def emit():
                            for idx, j in enumerate(js):
                                qoff = max(0, P * (j - 4 * iq))
                                nc.tensor.matmul(
                                    po[0:E2, qoff:512],
                                    lhsT=vp_t[:, j, :],
                                    rhs=at[:, idx, qoff:512],
                                    start=(j == 0),
                                    stop=(j == njs - 1),
                                )
                            if js[-1] != njs - 1:
                                return
                            # q-superblock epilogue: invert the denominator
                            # row (~51 ULP, way below fp16 noise), broadcast
                            # across partitions, normalize, store fp16.
                            rr = stg.tile([1, 512], f32, tag="rr")
                            nc.vector.reciprocal_approx_fast(
                                rr[:], po[0:1, :]
                            )
                            rb = stg.tile([E2, 512], f32, tag="rb")
                            nc.gpsimd.partition_broadcast(rb[:], rr[0:1, :])
                            outF = stg.tile([E2, 512], fp16, tag="outF")
                            nc.vector.tensor_tensor(
                                outF[0:E2, :], po[0:E2, :], rb[0:E2, :], Mult
                            )
                            nc.sync.dma_start(
                                o_d[p, :, q0 : q0 + 512], outF[1:E2, :]
                            )

                        return emit

                    # group schedule: off-diagonal chunks in 3s, then the 4
                    # diagonal chunks as [d0,d1],[d2,d3] (tight qmin).
                    groups = []
                    noff = 4 * iq
                    for g0 in range(0, noff, EXP_GROUP):
                        groups.append(list(range(g0, min(g0 + EXP_GROUP, noff))))
                    groups.append([noff, noff + 1])
                    groups.append([noff + 2, noff + 3])

                    for js in groups:
                        ng = len(js)
                        is_diag = js[0] >= noff
                        ps = psS.tile([P, EXP_GROUP, 512], f32, tag="ps")
                        for idx, j in enumerate(js):
                            row = 64 * (j % 2)  # alternate row groups: LDW hides
                            qoff = max(0, P * (j - 4 * iq))
                            nc.tensor.matmul(
                                ps[:, idx, qoff:512],
                                lhsT=kt_t[row : row + 64, P * j : P * (j + 1)],
                                rhs=qt_t[row : row + 64, q0 + qoff : q0 + 512],
                                start=True,
                                stop=True,
                            )
                        at = atp.tile([P, EXP_GROUP, 512], fp16, tag="at")
                        # exp whole group; skip columns no chunk needs
                        qmin = min(max(0, P * (j - 4 * iq)) for j in js)
                        w = ng * (512 - qmin)
                        cost_act = 0.833 * w + 185.0
                        cost_dve = 1.0417 * w + 125.0
                        use_dve = (not is_diag) and (
                            eng_ns["dve"] + cost_dve < eng_ns["act"] + cost_act
                        )
                        if use_dve:
                            eng_ns["dve"] += cost_dve
                            at_i = at.bitcast(i16)
                            nc.vector.tensor_scalar(
                                at_i[:, :ng, qmin:512],
                                ps[:, :ng, qmin:512],
                                sck_t[:, p : p + 1],
                                M16,
                                Mult,
                                Add,
                            )
                        else:
                            eng_ns["act"] += cost_act
                            nc.scalar.activation(
                                at[:, :ng, qmin:512],
                                ps[:, :ng, qmin:512],
                                Exp,
                                scale=scl_t[:, p : p + 1],
                            )
                        if is_diag:
                            for idx, j in enumerate(js):
                                qo = P * (j - noff)  # diag tile offset
                                meng = nc.vector if p == 0 else nc.gpsimd
                                meng.tensor_tensor(
                                    at[:, idx, qo : qo + P],
                                    at[:, idx, qo : qo + P],
                                    tri_t[:],
                                    Mult,
                                )
                        flush(depth=1)
                        pending.append(mk_mm2(js, at))
            flush()

    nc.compile()
    _PROGRAM_CACHE["nc"] = nc
    return nc


def _prep_core_inputs(queries, keys, values, tau, core):
    qt = np.empty((PAIRS_PER_CORE, P, L), dtype=np.float16)
    kt = np.empty((PAIRS_PER_CORE, P, L), dtype=np.float16)
    vp = np.zeros((PAIRS_PER_CORE, P, NSC, E2), dtype=np.float16)
    scl = np.empty((P, PAIRS_PER_CORE), dtype=np.float32)
    sck = np.empty((P, PAIRS_PER_CORE), dtype=np.float32)
    for p in range(PAIRS_PER_CORE):
        idx = PAIRS_PER_CORE * core + p
        b, h = divmod(idx, H)
        qT = np.ascontiguousarray(queries[b, :, h, :].T).astype(np.float16)  # [E, L]
        kT = np.ascontiguousarray(keys[b, :, h, :].T).astype(np.float16)
        qt[p, 0:E] = qT
        qt[p, E:P] = qT
        kt[p, 0:E] = kT
        kt[p, E:P] = kT
        # vp[p, si, so, 1+e] = V[b, 128*so + si, h, e]; ones in column 0
        vv = values[b, :, h, :].reshape(NSC, P, E).transpose(1, 0, 2)
        vp[p, :, :, 1 : E + 1] = vv.astype(np.float16)
        vp[p, :, :, 0] = 1.0
        s = np.exp(tau[b, 0, 0, 0]) / np.sqrt(E)
        scl[:, p] = s
        sck[:, p] = s * K16
    tri = np.triu(np.ones((P, P), dtype=np.float16))  # tri[s, q] = 1 iff s <= q
    return {"qt": qt, "kt": kt, "vp": vp, "tri": tri, "scl": scl, "sck": sck}


def _run(inputs, trace=False):
    queries = np.asarray(inputs["queries"], dtype=np.float32)
    keys = np.asarray(inputs["keys"], dtype=np.float32)
    values = np.asarray(inputs["values"], dtype=np.float32)
    tau = np.asarray(inputs["tau"], dtype=np.float32)

    nc = _build_program()
    in_maps = [
        _prep_core_inputs(queries, keys, values, tau, c) for c in range(NCORES)
    ]
    res = bass_utils.run_bass_kernel_spmd(
        nc, in_maps, core_ids=list(range(NCORES)), trace=trace
    )
    out = np.empty((B, L, H, E), dtype=np.float32)
    for c in range(NCORES):
        o = res.results[c]["o"]  # [PAIRS, E, L] fp16
        for p in range(PAIRS_PER_CORE):
            idx = PAIRS_PER_CORE * c + p
            b, h = divmod(idx, H)
            out[b, :, h, :] = o[p].T.astype(np.float32)
    return out, res


def kernel(queries, keys, values, attn_mask, tau):
    out, _ = _run(
        {"queries": queries, "keys": keys, "values": values, "tau": tau},
        trace=False,
    )
    return out


def kernel_traced(queries, keys, values, attn_mask, tau):
    out, res = _run(
        {"queries": queries, "keys": keys, "values": values, "tau": tau},
        trace=True,
    )
    return out, res


# revision 10
# speedup vs baseline: 1.5445x; 1.5445x over previous
"""Causal attention (DS_FullAttention) Trainium2 Bass kernel.

Problem: B=4, H=8, L=S=2048, E=64 causal attention with a per-batch
exp(tau) de-stationarization rescale, fp32 I/O.

Sharding: the 32 (b, h) pairs are independent; each of the 8 cores gets 4
pairs. Inside a core, per pair:
  - scoresT[s, q] = K^T Q computed transposed (s on PSUM partitions) so the
    A @ V contraction (over s) needs no on-chip transpose of A.
  - softmax exp is SPLIT across two engines: the scalar (ACT) engine runs
    table-based Exp with the per-pair scale folded into the ACTIVATE scale
    operand; the vector (DVE) engine runs a one-op Schraudolph exp
    (int16 = score * K + M, bits reinterpreted as fp16) for a tunable
    share of the off-diagonal chunks.  Max-subtraction is skipped; scores
    are O(8) so fp16 exp is safe.
  - V carries an appended ones column, so the A @ V matmul also produces
    the softmax denominators in PSUM partition 0.
  - causal masking: block-level (never compute s-chunks above the
    diagonal) + a triangular fp16 mask multiply on diagonal 128x128 chunks
    (gpsimd engine; DVE for pair 0 while gpsimd warms up).
  - normalization: reciprocal of the PSUM denominator row on DVE,
    partition-broadcast on gpsimd, one PSUM-sourced multiply on DVE
    writing fp16, fp16 output DMA (host upcasts to fp32).

Matmuls run in fp16 (measured end-to-end error ~5e-4 vs the fp32
reference; fp16 hides PE weight loads, fp32/f32r cannot).
"""

import sys

if "/opt/trn_rl_repo" not in sys.path:
    sys.path.insert(0, "/opt/trn_rl_repo")

import numpy as np

import concourse.bass as bass
import concourse.mybir as mybir
import concourse.tile as tile
from concourse import bacc, bass_utils

B, L, S, H, E = 4, 2048, 2048, 8, 64
P = 128
NCORES = 8
PAIRS_PER_CORE = (B * H) // NCORES  # 4
NQB = L // 512  # 4 q-superblocks of 512
NSC = S // P  # 16 s-chunks of 128
E2 = E + 1  # V plus ones column
EXP_GROUP = 3  # s-chunks exp'd per instruction (3 PSUM banks)

# Schraudolph fp16 exp constants: i16 = round(s*scale*K16 + M16); bits as
# fp16 give exp(s*scale) with ~3% max rel err (C=44 minimizes max rel err).
K16 = 2.0**10 / np.log(2.0)
M16 = 15.0 * 2**10 - 44.0
# Epilogue reciprocal-row broadcast: "dma" (sync-engine stride-0 DMA; keeps
# gpsimd single-library) or "gpsimd" (PartitionBroadcast; masks then move to
# DVE to avoid Q7 library thrash).
BCAST_MODE = "dma"

f32 = mybir.dt.float32
fp16 = mybir.dt.float16
i16 = mybir.dt.int16
Exp = mybir.ActivationFunctionType.Exp
Mult = mybir.AluOpType.mult
Add = mybir.AluOpType.add

_PROGRAM_CACHE = {}


def _build_program():
    if "nc" in _PROGRAM_CACHE:
        return _PROGRAM_CACHE["nc"]

    nc = bacc.Bacc(
        "TRN2",
        target_bir_lowering=False,
        debug=False,
        enable_asserts=False,
        num_devices=NCORES,
    )
    qt_d = nc.dram_tensor("qt", [PAIRS_PER_CORE, P, L], fp16, kind="ExternalInput")
    kt_d = nc.dram_tensor("kt", [PAIRS_PER_CORE, P, L], fp16, kind="ExternalInput")
    vp_d = nc.dram_tensor(
        "vp", [PAIRS_PER_CORE, P, NSC, E2], fp16, kind="ExternalInput"
    )
    tri_d = nc.dram_tensor("tri", [P, P], fp16, kind="ExternalInput")
    scl_d = nc.dram_tensor("scl", [P, PAIRS_PER_CORE], f32, kind="ExternalInput")
    sck_d = nc.dram_tensor("sck", [P, PAIRS_PER_CORE], f32, kind="ExternalInput")
    o_d = nc.dram_tensor("o", [PAIRS_PER_CORE, E, L], fp16, kind="ExternalOutput")

    with tile.TileContext(nc) as tc:
        with (
            tc.tile_pool(name="const", bufs=1) as const,
            tc.tile_pool(name="qk", bufs=2) as qk,
            tc.tile_pool(name="atp", bufs=5) as atp,
            tc.tile_pool(name="stg", bufs=3) as stg,
            tc.tile_pool(name="psS", bufs=2, space="PSUM") as psS,
            tc.tile_pool(name="psO", bufs=2, space="PSUM") as psO,
        ):
            # constants first so their DMAs lead the queue
            tri_t = const.tile([P, P], fp16)
            nc.sync.dma_start(tri_t[:], tri_d[:])
            scl_t = const.tile([P, PAIRS_PER_CORE], f32)
            nc.sync.dma_start(scl_t[:], scl_d[:])
            sck_t = const.tile([P, PAIRS_PER_CORE], f32)
            nc.sync.dma_start(sck_t[:], sck_d[:])

            # ACT warm-up: pull the exp table load under the input DMAs
            # (no gpsimd dependency -- ACT zeroes its own scratch).
            wu = const.tile([P, 16], f32)
            nc.scalar.memzero(wu[:])
            nc.scalar.activation(wu[:], wu[:], Exp, scale=1.0)
            # gpsimd warm-up: the Q7 engine DMA-fetches a ucode library per
            # op TYPE (a switch costs ~6us) -- run gpsimd as a single-op
            # (TensorTensor Multiply) engine and load that library now.
            wg = const.tile([P, 16], fp16)
            nc.gpsimd.tensor_tensor(wg[:], tri_t[:, 0:16], tri_t[:, 0:16], Mult)

            # mm2 + block epilogues run two exp-groups behind (RAW decoupling)
            pending = []

            def flush(depth=0):
                while len(pending) > depth:
                    pending.pop(0)()

            # greedy ACT/DVE off-diagonal exp balancing (estimated busy ns);
            # DVE starts pre-loaded with its epilogue (and mask) work.
            dve_seed = 21000.0 if BCAST_MODE == "dma" else 21000.0 + 12800.0
            eng_ns = {"act": 0.0, "dve": dve_seed}

            for p in range(PAIRS_PER_CORE):
                qt_t = qk.tile([P, L], fp16, tag="qt")
                kt_t = qk.tile([P, L], fp16, tag="kt")
                vp_t = qk.tile([P, NSC, E2], fp16, tag="vp")
                if p == 0:  # small first slices so group 0 starts early
                    nc.sync.dma_start(kt_t[:, 0:512], kt_d[p][:, 0:512])
                    nc.sync.dma_start(qt_t[:, 0:512], qt_d[p][:, 0:512])
                    nc.sync.dma_start(vp_t[:, 0:4, :], vp_d[p][:, 0:4, :])
                    nc.sync.dma_start(kt_t[:, 512:L], kt_d[p][:, 512:L])
                    nc.sync.dma_start(qt_t[:, 512:L], qt_d[p][:, 512:L])
                    nc.sync.dma_start(vp_t[:, 4:NSC, :], vp_d[p][:, 4:NSC, :])
                else:
                    nc.sync.dma_start(qt_t[:], qt_d[p])
                    nc.sync.dma_start(kt_t[:], kt_d[p])
                    nc.sync.dma_start(vp_t[:], vp_d[p])

                # last pair runs its q-superblocks big-to-small so the tail
                # ends on the cheapest superblock (iq=0).
                iq_order = (
                    range(NQB) if p < PAIRS_PER_CORE - 1 else range(NQB - 1, -1, -1)
                )
                for iq in iq_order:
                    q0 = 512 * iq
                    njs = 4 * iq + 4  # s-chunks this q-superblock touches
                    po = psO.tile([P, 512], f32, tag="po")

                    def mk_mm2(js, at, po=po, vp_t=vp_t, iq=iq, njs=njs,
                               p=p, q0=q0):
                        def emit():
                            for idx, j in enumerate(js):
                                qoff = max(0, P * (j - 4 * iq))
                                nc.tensor.matmul(
                                    po[0:E2, qoff:512],
                                    lhsT=vp_t[:, j, :],
                                    rhs=at[:, idx, qoff:512],
                                    start=(j == 0),
                                    stop=(j == njs - 1),
                                )
                            if js[-1] != njs - 1:
                                return
                            # q-superblock epilogue: invert the denominator
                            # row (~51 ULP, way below fp16 noise), broadcast
                            # across partitions, normalize, store fp16.
                            rr = stg.tile([1, 512], f32, tag="rr")
                            nc.vector.reciprocal_approx_fast(
                                rr[:], po[0:1, :]
                            )
                            rb = stg.tile([E2, 512], f32, tag="rb")
                            if BCAST_MODE == "dma":
                                src = rr[0:1, :]
                                bsrc = bass.AP(
                                    src.tensor, src.offset,
                                    [[1, 1], [0, E2], [1, 512]],
                                )
                                nc.sync.dma_start(rb[:], bsrc)
                            else:
                                nc.gpsimd.partition_broadcast(
                                    rb[:], rr[0:1, :]
                                )
                            outF = stg.tile([E2, 512], fp16, tag="outF")
                            nc.vector.tensor_tensor(
                                outF[0:E2, :], po[0:E2, :], rb[0:E2, :], Mult
                            )
                            nc.sync.dma_start(
                                o_d[p, :, q0 : q0 + 512], outF[1:E2, :]
                            )

                        return emit

                    # group schedule: off-diagonal chunks in 3s, then the 4
                    # diagonal chunks as [d0,d1],[d2,d3] (tight qmin).
                    groups = []
                    noff = 4 * iq
                    for g0 in range(0, noff, EXP_GROUP):
                        groups.append(list(range(g0, min(g0 + EXP_GROUP, noff))))
                    groups.append([noff, noff + 1])
                    groups.append([noff + 2, noff + 3])

                    for js in groups:
                        ng = len(js)
                        is_diag = js[0] >= noff
                        ps = psS.tile([P, EXP_GROUP, 512], f32, tag="ps")
                        for idx, j in enumerate(js):
                            row = 64 * (j % 2)  # alternate row groups: LDW hides
                            qoff = max(0, P * (j - 4 * iq))
                            nc.tensor.matmul(
                                ps[:, idx, qoff:512],
                                lhsT=kt_t[row : row + 64, P * j : P * (j + 1)],
                                rhs=qt_t[row : row + 64, q0 + qoff : q0 + 512],
                                start=True,
                                stop=True,
                            )
                        at = atp.tile([P, EXP_GROUP, 512], fp16, tag="at")
                        # exp whole group; skip columns no chunk needs
                        qmin = min(max(0, P * (j - 4 * iq)) for j in js)
                        w = ng * (512 - qmin)
                        cost_act = 0.833 * w + 185.0
                        cost_dve = 1.0417 * w + 125.0
                        use_dve = (not is_diag) and (
                            eng_ns["dve"] + cost_dve < eng_ns["act"] + cost_act
                        )
                        if use_dve:
                            eng_ns["dve"] += cost_dve
                            at_i = at.bitcast(i16)
                            nc.vector.tensor_scalar(
                                at_i[:, :ng, qmin:512],
                                ps[:, :ng, qmin:512],
                                sck_t[:, p : p + 1],
                                M16,
                                Mult,
                                Add,
                            )
                        else:
                            eng_ns["act"] += cost_act
                            nc.scalar.activation(
                                at[:, :ng, qmin:512],
                                ps[:, :ng, qmin:512],
                                Exp,
                                scale=scl_t[:, p : p + 1],
                            )
                        if is_diag:
                            for idx, j in enumerate(js):
                                qo = P * (j - noff)  # diag tile offset
                                meng = (
                                    nc.gpsimd
                                    if BCAST_MODE == "dma"
                                    else nc.vector
                                )
                                meng.tensor_tensor(
                                    at[:, idx, qo : qo + P],
                                    at[:, idx, qo : qo + P],
                                    tri_t[:],
                                    Mult,
                                )
                        flush(depth=1)
                        pending.append(mk_mm2(js, at))
            flush()

    nc.compile()
    _PROGRAM_CACHE["nc"] = nc
    return nc


def _prep_core_inputs(queries, keys, values, tau, core):
    qt = np.empty((PAIRS_PER_CORE, P, L), dtype=np.float16)
    kt = np.empty((PAIRS_PER_CORE, P, L), dtype=np.float16)
    vp = np.zeros((PAIRS_PER_CORE, P, NSC, E2), dtype=np.float16)
    scl = np.empty((P, PAIRS_PER_CORE), dtype=np.float32)
    sck = np.empty((P, PAIRS_PER_CORE), dtype=np.float32)
    for p in range(PAIRS_PER_CORE):
        idx = PAIRS_PER_CORE * core + p
        b, h = divmod(idx, H)
        qT = np.ascontiguousarray(queries[b, :, h, :].T).astype(np.float16)  # [E, L]
        kT = np.ascontiguousarray(keys[b, :, h, :].T).astype(np.float16)
        qt[p, 0:E] = qT
        qt[p, E:P] = qT
        kt[p, 0:E] = kT
        kt[p, E:P] = kT
        # vp[p, si, so, 1+e] = V[b, 128*so + si, h, e]; ones in column 0
        vv = values[b, :, h, :].reshape(NSC, P, E).transpose(1, 0, 2)
        vp[p, :, :, 1 : E + 1] = vv.astype(np.float16)
        vp[p, :, :, 0] = 1.0
        s = np.exp(tau[b, 0, 0, 0]) / np.sqrt(E)
        scl[:, p] = s
        sck[:, p] = s * K16
    tri = np.triu(np.ones((P, P), dtype=np.float16))  # tri[s, q] = 1 iff s <= q
    return {"qt": qt, "kt": kt, "vp": vp, "tri": tri, "scl": scl, "sck": sck}


def _run(inputs, trace=False):
    queries = np.asarray(inputs["queries"], dtype=np.float32)
    keys = np.asarray(inputs["keys"], dtype=np.float32)
    values = np.asarray(inputs["values"], dtype=np.float32)
    tau = np.asarray(inputs["tau"], dtype=np.float32)

    nc = _build_program()
    in_maps = [
        _prep_core_inputs(queries, keys, values, tau, c) for c in range(NCORES)
    ]
    res = bass_utils.run_bass_kernel_spmd(
        nc, in_maps, core_ids=list(range(NCORES)), trace=trace
    )
    out = np.empty((B, L, H, E), dtype=np.float32)
    for c in range(NCORES):
        o = res.results[c]["o"]  # [PAIRS, E, L] fp16
        for p in range(PAIRS_PER_CORE):
            idx = PAIRS_PER_CORE * c + p
            b, h = divmod(idx, H)
            out[b, :, h, :] = o[p].T.astype(np.float32)
    return out, res


def kernel(queries, keys, values, attn_mask, tau):
    out, _ = _run(
        {"queries": queries, "keys": keys, "values": values, "tau": tau},
        trace=False,
    )
    return out


def kernel_traced(queries, keys, values, attn_mask, tau):
    out, res = _run(
        {"queries": queries, "keys": keys, "values": values, "tau": tau},
        trace=True,
    )
    return out, res


# revision 12
# speedup vs baseline: 2.1944x; 1.4207x over previous
"""Causal attention (DS_FullAttention) Trainium2 Bass kernel.

Problem: B=4, H=8, L=S=2048, E=64 causal attention with a per-batch
exp(tau) de-stationarization rescale, fp32 I/O.

Sharding: the 32 (b, h) pairs are independent; each of the 8 cores gets 4
pairs. Inside a core, per pair:
  - scoresT[s, q] = K^T Q computed transposed (s on PSUM partitions) so the
    A @ V contraction (over s) needs no on-chip transpose of A.
  - softmax exp is SPLIT across two engines: the scalar (ACT) engine runs
    table-based Exp with the per-pair scale folded into the ACTIVATE scale
    operand; the vector (DVE) engine runs a one-op Schraudolph exp
    (int16 = score * K + M, bits reinterpreted as fp16) for a tunable
    share of the off-diagonal chunks.  Max-subtraction is skipped; scores
    are O(8) so fp16 exp is safe.
  - V carries an appended ones column, so the A @ V matmul also produces
    the softmax denominators in PSUM partition 0.
  - causal masking: block-level (never compute s-chunks above the
    diagonal) + a triangular fp16 mask multiply on diagonal 128x128 chunks
    (gpsimd engine; DVE for pair 0 while gpsimd warms up).
  - normalization: reciprocal of the PSUM denominator row on DVE,
    partition-broadcast on gpsimd, one PSUM-sourced multiply on DVE
    writing fp16, fp16 output DMA (host upcasts to fp32).

Matmuls run in fp16 (measured end-to-end error ~5e-4 vs the fp32
reference; fp16 hides PE weight loads, fp32/f32r cannot).
"""

import sys

if "/opt/trn_rl_repo" not in sys.path:
    sys.path.insert(0, "/opt/trn_rl_repo")

import numpy as np

import concourse.bass as bass
import concourse.mybir as mybir
import concourse.tile as tile
from concourse import bacc, bass_utils

B, L, S, H, E = 4, 2048, 2048, 8, 64
P = 128
NCORES = 8
PAIRS_PER_CORE = (B * H) // NCORES  # 4
NQB = L // 512  # 4 q-superblocks of 512
NSC = S // P  # 16 s-chunks of 128
E2 = E + 1  # V plus ones column
EXP_GROUP = 3  # s-chunks exp'd per instruction (3 PSUM banks)

# Schraudolph fp16 exp constants: i16 = round(s*scale*K16 + M16); bits as
# fp16 give exp(s*scale) with ~3% max rel err (C=44 minimizes max rel err).
K16 = 2.0**10 / np.log(2.0)
M16 = 15.0 * 2**10 - 44.0
# Epilogue reciprocal-row broadcast: "dma" (sync-engine stride-0 DMA -- dead
# end: 65 descriptors reading ONE SBUF partition bottleneck on its read port)
# or "gpsimd" (PartitionBroadcast; masks then live on DVE so gpsimd runs a
# single op type -- a Q7 library switch costs ~6us).
BCAST_MODE = "gpsimd"

f32 = mybir.dt.float32
fp16 = mybir.dt.float16
i16 = mybir.dt.int16
Exp = mybir.ActivationFunctionType.Exp
Mult = mybir.AluOpType.mult
Add = mybir.AluOpType.add

_PROGRAM_CACHE = {}


def _build_program():
    if "nc" in _PROGRAM_CACHE:
        return _PROGRAM_CACHE["nc"]

    nc = bacc.Bacc(
        "TRN2",
        target_bir_lowering=False,
        debug=False,
        enable_asserts=False,
        num_devices=NCORES,
    )
    qt_d = nc.dram_tensor("qt", [PAIRS_PER_CORE, P, L], fp16, kind="ExternalInput")
    kt_d = nc.dram_tensor("kt", [PAIRS_PER_CORE, P, L], fp16, kind="ExternalInput")
    vp_d = nc.dram_tensor(
        "vp", [PAIRS_PER_CORE, P, NSC, E2], fp16, kind="ExternalInput"
    )
    tri_d = nc.dram_tensor("tri", [P, P], fp16, kind="ExternalInput")
    scl_d = nc.dram_tensor("scl", [P, PAIRS_PER_CORE], f32, kind="ExternalInput")
    sck_d = nc.dram_tensor("sck", [P, PAIRS_PER_CORE], f32, kind="ExternalInput")
    o_d = nc.dram_tensor("o", [PAIRS_PER_CORE, E, L], fp16, kind="ExternalOutput")

    with tile.TileContext(nc) as tc:
        with (
            tc.tile_pool(name="const", bufs=1) as const,
            tc.tile_pool(name="qk", bufs=2) as qk,
            tc.tile_pool(name="atp", bufs=5) as atp,
            tc.tile_pool(name="stg", bufs=3) as stg,
            tc.tile_pool(name="psS", bufs=2, space="PSUM") as psS,
            tc.tile_pool(name="psO", bufs=2, space="PSUM") as psO,
        ):
            # constants first so their DMAs lead the queue
            tri_t = const.tile([P, P], fp16)
            nc.sync.dma_start(tri_t[:], tri_d[:])
            scl_t = const.tile([P, PAIRS_PER_CORE], f32)
            nc.sync.dma_start(scl_t[:], scl_d[:])
            sck_t = const.tile([P, PAIRS_PER_CORE], f32)
            nc.sync.dma_start(sck_t[:], sck_d[:])

            # ACT warm-up: pull the exp table load under the input DMAs
            # (no gpsimd dependency -- ACT zeroes its own scratch).
            wu = const.tile([P, 16], f32)
            nc.scalar.memzero(wu[:])
            nc.scalar.activation(wu[:], wu[:], Exp, scale=1.0)
            # gpsimd warm-up: the Q7 engine DMA-fetches a ucode library per
            # op TYPE (a switch costs ~6us) -- run gpsimd as a single-op
            # (PartitionBroadcast) engine and load that library now.
            wg = const.tile([P, 16], f32)
            nc.gpsimd.partition_broadcast(wg[0:2, :], wu[0:1, :])

            # mm2 + block epilogues run two exp-groups behind (RAW decoupling)
            pending = []

            def flush(depth=0):
                while len(pending) > depth:
                    pending.pop(0)()

            # greedy ACT/DVE off-diagonal exp balancing (estimated busy ns);
            # DVE starts pre-loaded with its epilogue (and mask) work.
            dve_seed = 21000.0 if BCAST_MODE == "dma" else 21000.0 + 12800.0
            eng_ns = {"act": 0.0, "dve": dve_seed}

            for p in range(PAIRS_PER_CORE):
                qt_t = qk.tile([P, L], fp16, tag="qt")
                kt_t = qk.tile([P, L], fp16, tag="kt")
                vp_t = qk.tile([P, NSC, E2], fp16, tag="vp")
                if p == 0:  # small first slices so group 0 starts early
                    nc.sync.dma_start(kt_t[:, 0:512], kt_d[p][:, 0:512])
                    nc.sync.dma_start(qt_t[:, 0:512], qt_d[p][:, 0:512])
                    nc.sync.dma_start(vp_t[:, 0:4, :], vp_d[p][:, 0:4, :])
                    nc.sync.dma_start(kt_t[:, 512:L], kt_d[p][:, 512:L])
                    nc.sync.dma_start(qt_t[:, 512:L], qt_d[p][:, 512:L])
                    nc.sync.dma_start(vp_t[:, 4:NSC, :], vp_d[p][:, 4:NSC, :])
                else:
                    nc.sync.dma_start(qt_t[:], qt_d[p])
                    nc.sync.dma_start(kt_t[:], kt_d[p])
                    nc.sync.dma_start(vp_t[:], vp_d[p])

                # last pair runs its q-superblocks big-to-small so the tail
                # ends on the cheapest superblock (iq=0).
                iq_order = (
                    range(NQB) if p < PAIRS_PER_CORE - 1 else range(NQB - 1, -1, -1)
                )
                for iq in iq_order:
                    q0 = 512 * iq
                    njs = 4 * iq + 4  # s-chunks this q-superblock touches
                    po = psO.tile([P, 512], f32, tag="po")

                    def mk_mm2(js, at, po=po, vp_t=vp_t, iq=iq, njs=njs,
                               p=p, q0=q0):
                        def emit():
                            for idx, j in enumerate(js):
                                qoff = max(0, P * (j - 4 * iq))
                                nc.tensor.matmul(
                                    po[0:E2, qoff:512],
                                    lhsT=vp_t[:, j, :],
                                    rhs=at[:, idx, qoff:512],
                                    start=(j == 0),
                                    stop=(j == njs - 1),
                                )
                            if js[-1] != njs - 1:
                                return
                            # q-superblock epilogue: invert the denominator
                            # row (~51 ULP, way below fp16 noise), broadcast
                            # across partitions, normalize, store fp16.
                            rr = stg.tile([1, 512], f32, tag="rr")
                            nc.vector.reciprocal_approx_fast(
                                rr[:], po[0:1, :]
                            )
                            rb = stg.tile([E2, 512], f32, tag="rb")
                            if BCAST_MODE == "dma":
                                src = rr[0:1, :]
                                bsrc = bass.AP(
                                    src.tensor, src.offset,
                                    [[1, 1], [0, E2], [1, 512]],
                                )
                                nc.sync.dma_start(rb[:], bsrc)
                            else:
                                nc.gpsimd.partition_broadcast(
                                    rb[:], rr[0:1, :]
                                )
                            outF = stg.tile([E2, 512], fp16, tag="outF")
                            nc.vector.tensor_tensor(
                                outF[0:E2, :], po[0:E2, :], rb[0:E2, :], Mult
                            )
                            nc.sync.dma_start(
                                o_d[p, :, q0 : q0 + 512], outF[1:E2, :]
                            )

                        return emit

                    # group schedule: off-diagonal chunks in 3s, then the 4
                    # diagonal chunks as [d0,d1],[d2,d3] (tight qmin).
                    groups = []
                    noff = 4 * iq
                    for g0 in range(0, noff, EXP_GROUP):
                        groups.append(list(range(g0, min(g0 + EXP_GROUP, noff))))
                    groups.append([noff, noff + 1])
                    groups.append([noff + 2, noff + 3])

                    for js in groups:
                        ng = len(js)
                        is_diag = js[0] >= noff
                        ps = psS.tile([P, EXP_GROUP, 512], f32, tag="ps")
                        for idx, j in enumerate(js):
                            row = 64 * (j % 2)  # alternate row groups: LDW hides
                            qoff = max(0, P * (j - 4 * iq))
                            nc.tensor.matmul(
                                ps[:, idx, qoff:512],
                                lhsT=kt_t[row : row + 64, P * j : P * (j + 1)],
                                rhs=qt_t[row : row + 64, q0 + qoff : q0 + 512],
                                start=True,
                                stop=True,
                            )
                        at = atp.tile([P, EXP_GROUP, 512], fp16, tag="at")
                        # exp whole group; skip columns no chunk needs
                        qmin = min(max(0, P * (j - 4 * iq)) for j in js)
                        w = ng * (512 - qmin)
                        cost_act = 0.833 * w + 185.0
                        cost_dve = 1.0417 * w + 125.0
                        use_dve = (not is_diag) and (
                            eng_ns["dve"] + cost_dve < eng_ns["act"] + cost_act
                        )
                        if use_dve:
                            eng_ns["dve"] += cost_dve
                            at_i = at.bitcast(i16)
                            nc.vector.tensor_scalar(
                                at_i[:, :ng, qmin:512],
                                ps[:, :ng, qmin:512],
                                sck_t[:, p : p + 1],
                                M16,
                                Mult,
                                Add,
                            )
                        else:
                            eng_ns["act"] += cost_act
                            nc.scalar.activation(
                                at[:, :ng, qmin:512],
                                ps[:, :ng, qmin:512],
                                Exp,
                                scale=scl_t[:, p : p + 1],
                            )
                        if is_diag:
                            for idx, j in enumerate(js):
                                qo = P * (j - noff)  # diag tile offset
                                meng = (
                                    nc.gpsimd
                                    if BCAST_MODE == "dma"
                                    else nc.vector
                                )
                                meng.tensor_tensor(
                                    at[:, idx, qo : qo + P],
                                    at[:, idx, qo : qo + P],
                                    tri_t[:],
                                    Mult,
                                )
                        flush(depth=1)
                        pending.append(mk_mm2(js, at))
            flush()

    nc.compile()
    _PROGRAM_CACHE["nc"] = nc
    return nc


def _prep_core_inputs(queries, keys, values, tau, core):
    qt = np.empty((PAIRS_PER_CORE, P, L), dtype=np.float16)
    kt = np.empty((PAIRS_PER_CORE, P, L), dtype=np.float16)
    vp = np.zeros((PAIRS_PER_CORE, P, NSC, E2), dtype=np.float16)
    scl = np.empty((P, PAIRS_PER_CORE), dtype=np.float32)
    sck = np.empty((P, PAIRS_PER_CORE), dtype=np.float32)
    for p in range(PAIRS_PER_CORE):
        idx = PAIRS_PER_CORE * core + p
        b, h = divmod(idx, H)
        qT = np.ascontiguousarray(queries[b, :, h, :].T).astype(np.float16)  # [E, L]
        kT = np.ascontiguousarray(keys[b, :, h, :].T).astype(np.float16)
        qt[p, 0:E] = qT
        qt[p, E:P] = qT
        kt[p, 0:E] = kT
        kt[p, E:P] = kT
        # vp[p, si, so, 1+e] = V[b, 128*so + si, h, e]; ones in column 0
        vv = values[b, :, h, :].reshape(NSC, P, E).transpose(1, 0, 2)
        vp[p, :, :, 1 : E + 1] = vv.astype(np.float16)
        vp[p, :, :, 0] = 1.0
        s = np.exp(tau[b, 0, 0, 0]) / np.sqrt(E)
        scl[:, p] = s
        sck[:, p] = s * K16
    tri = np.triu(np.ones((P, P), dtype=np.float16))  # tri[s, q] = 1 iff s <= q
    return {"qt": qt, "kt": kt, "vp": vp, "tri": tri, "scl": scl, "sck": sck}


def _run(inputs, trace=False):
    queries = np.asarray(inputs["queries"], dtype=np.float32)
    keys = np.asarray(inputs["keys"], dtype=np.float32)
    values = np.asarray(inputs["values"], dtype=np.float32)
    tau = np.asarray(inputs["tau"], dtype=np.float32)

    nc = _build_program()
    in_maps = [
        _prep_core_inputs(queries, keys, values, tau, c) for c in range(NCORES)
    ]
    res = bass_utils.run_bass_kernel_spmd(
        nc, in_maps, core_ids=list(range(NCORES)), trace=trace
    )
    out = np.empty((B, L, H, E), dtype=np.float32)
    for c in range(NCORES):
        o = res.results[c]["o"]  # [PAIRS, E, L] fp16
        for p in range(PAIRS_PER_CORE):
            idx = PAIRS_PER_CORE * c + p
            b, h = divmod(idx, H)
            out[b, :, h, :] = o[p].T.astype(np.float32)
    return out, res


def kernel(queries, keys, values, attn_mask, tau):
    out, _ = _run(
        {"queries": queries, "keys": keys, "values": values, "tau": tau},
        trace=False,
    )
    return out


def kernel_traced(queries, keys, values, attn_mask, tau):
    out, res = _run(
        {"queries": queries, "keys": keys, "values": values, "tau": tau},
        trace=True,
    )
    return out, res
